# revision 2
# baseline (speedup 1.0000x reference)
"""Bilinear interpolation (affine grid sample) TRN2 Bass kernel, v5.

v5 = v4 + data-dependent chunk skipping:
  - Points are enumerated in 32(iy) x 64(ix) spatial tiles (32 chunks of
    2048/batch).  Host computes per-chunk "any maybe-valid point" flags
    from theta (conservative eps=0.01 margin); fully-out-of-bounds chunks
    gather ZERO windows (num_idxs_reg loaded from an input tensor, idx=-1)
    while the combine still runs (weights=0 -> zero output).
  - Host pairs heavy+light batches per core to balance skipped work.

Changes vs v2 (kernel.py):
  - Phase A rebuilt: X loaded as [128=(rowparity,ch), k*256+x] bf16 slices;
    256 full-width [128,128] PE transposes per batch (vs 512 half-width);
    ONE ACT copy per row-pair P (vs 2); hs quarter tiles flushed by big
    strided DMA.
  - Phase E combine trimmed to 6 real slots (skip zero slots 3,7):
    3072*3 + 1024*2 DVE els/chunk vs 4096*3+2048+1024.
  - Program order A0 CD0 A1 E0 CD1 E1 so batch-1 build overlaps batch-0
    gather/combine; idx/weight pools double-buffered.
"""

import sys

sys.path.insert(0, "/opt/trn_rl_repo")

import numpy as np
import ml_dtypes  # noqa: F401

import concourse.bass as bass
import concourse.bacc as bacc
import concourse.mybir as mybir
from concourse import tile
from concourse.bass import AP
from concourse.masks import make_identity

F32 = mybir.dt.float32
BF16 = mybir.dt.bfloat16
I16 = mybir.dt.int16
I32 = mybir.dt.int32

B, C, H, W = 16, 64, 256, 256
HW = H * W
NCORES = 8
BPC = B // NCORES  # 2

NPTS = HW
NGRP = NPTS // 128  # 512
JCHUNK = 16
NCHUNK = NGRP // JCHUNK
CHUNK_PTS = JCHUNK * 128

NREC = 32768  # records of 256 bf16 el (512 B); u = (y0>>1)*256 + x0
RECEL = 256  # elements per record step
WINEL = 512  # gather window: 2 records = 8 px = 1 KB
GYSZ = 128 * 65536 + WINEL  # + pad so the u=32767 AP row stays in-bounds

SCALE = 2.0 * (W / 2) / (W - 1)  # 256/255

NQH = 16  # h-records per hs slice tile


TILE_IY, TILE_IX = 32, 64  # chunk = one spatial tile
TGRID_X = W // TILE_IX  # 4


def _i_to_ixiy(i):
    """gather/combine position i -> (ix, iy) under tile enumeration."""
    i = np.asarray(i)
    ci, local = i // CHUNK_PTS, i % CHUNK_PTS
    bi, bj = ci // TGRID_X, ci % TGRID_X
    iyl, ixl = local // TILE_IX, local % TILE_IX
    return bj * TILE_IX + ixl, bi * TILE_IY + iyl


def _host_consts():
    q = np.arange(128)
    s = np.arange(NPTS // 16)
    j = np.arange(NGRP)
    p = np.arange(128)
    # wrapped idx layout: gather position i = 16*s + (q%16)
    iw = 16 * s[None, :] + (q % 16)[:, None]
    ixw, iyw = _i_to_ixiy(iw)
    # weight layout: position i = j*128 + p
    pw = 128 * j[None, :] + p[:, None]
    pjx, pjy = _i_to_ixiy(pw)
    return {
        "IXW": np.ascontiguousarray(ixw.astype(np.float64), "bfloat16"),
        "IYW": np.ascontiguousarray(iyw.astype(np.float64), "bfloat16"),
        "PJX": np.ascontiguousarray(pjx.astype(np.float64), "bfloat16"),
        "PJY": np.ascontiguousarray(pjy.astype(np.float64), "bfloat16"),
    }


def _host_chunk_flags(theta):
    """Conservative per-chunk any-maybe-valid flags [B, NCHUNK] from theta."""
    th = np.asarray(theta, np.float64)
    Bn = th.shape[0]
    ixg = np.arange(W)
    iyg = np.arange(H)
    IXg, IYg = np.meshgrid(ixg, iyg, indexing="xy")  # [iy, ix]
    Sg = 2.0 / (W - 1)
    eps = 0.01
    flags = np.zeros((Bn, NCHUNK), np.int32)
    for b in range(Bn):
        t = th[b].reshape(2, 3)
        xc = IXg * Sg - 1.0
        yc = IYg * Sg - 1.0
        xs = (t[0, 0] * xc + t[0, 1] * yc + t[0, 2] + 1) * (W / 2)
        ys = (t[1, 0] * xc + t[1, 1] * yc + t[1, 2] + 1) * (H / 2)
        v = (xs > -eps) & (xs < W - 1 + eps) & (ys > -eps) & (ys < H - 1 + eps)
        tiles = v.reshape(H // TILE_IY, TILE_IY, W // TILE_IX, TILE_IX)
        anyv = tiles.any(axis=(1, 3))  # [bands, cols]
        flags[b] = anyv.reshape(-1).astype(np.int32)
    return flags


def build_nc(n_batches=BPC, n_chunks=NCHUNK, debug=False, jchunk=JCHUNK,
             gbufs=2, sp=False, gcall=1, nq=2, scratch=32768,
             phases="ACDE", nogather=False, nocombine=False, order="interleave",
             reps=1):
    nc = bacc.Bacc("TRN2", target_bir_lowering=False, debug=debug,
                   num_swdge_queues=nq, dynamic_dma_scratch_size=scratch)

    x_in = nc.declare_dram_parameter("X", [n_batches, C, HW], F32, isOutput=False)
    th_in = nc.declare_dram_parameter("THETA", [n_batches, 128, 6], F32, isOutput=False)
    ixw_in = nc.declare_dram_parameter("IXW", [128, NPTS // 16], BF16, isOutput=False)
    iyw_in = nc.declare_dram_parameter("IYW", [128, NPTS // 16], BF16, isOutput=False)
    pjx_in = nc.declare_dram_parameter("PJX", [128, NGRP], BF16, isOutput=False)
    pjy_in = nc.declare_dram_parameter("PJY", [128, NGRP], BF16, isOutput=False)
    chf_in = nc.declare_dram_parameter("CHFLAG", [n_batches, 128, NCHUNK], I32,
                                       isOutput=False)
    chc_in = nc.declare_dram_parameter("CHCNT", [n_batches, 1, NCHUNK], I32,
                                       isOutput=False)
    out_ext = nc.declare_dram_parameter("OUT", [n_batches, C, HW], F32, isOutput=True)

    gys = [nc.dram_tensor(f"gy{b}", [GYSZ], BF16) for b in range(n_batches)]

    _regcache = {}

    with tile.TileContext(nc) as tc:
        import contextlib

        with contextlib.ExitStack() as ctx:
            cpool = ctx.enter_context(tc.tile_pool(name="consts", bufs=1))
            xpool = ctx.enter_context(tc.tile_pool(name="xload", bufs=2))
            apsum = ctx.enter_context(tc.tile_pool(name="tpsum", bufs=2, space="PSUM"))
            hpool = ctx.enter_context(tc.tile_pool(name="hsbuf", bufs=2))
            wpool = ctx.enter_context(tc.tile_pool(name="weights", bufs=2))
            spool = ctx.enter_context(tc.tile_pool(name="scratch", bufs=1))
            gpool = ctx.enter_context(tc.tile_pool(name="gather", bufs=gbufs))
            mpool = ctx.enter_context(tc.tile_pool(name="mulbuf", bufs=1))
            opool = ctx.enter_context(tc.tile_pool(name="outbuf", bufs=2))
            opsum = ctx.enter_context(tc.tile_pool(name="opsum", bufs=2, space="PSUM"))

            identb = cpool.tile([128, 128], BF16)
            make_identity(nc, identb[:])
            ixw = cpool.tile([128, NPTS // 16], BF16)
            nc.sync.dma_start(out=ixw[:], in_=ixw_in.ap())
            iyw = cpool.tile([128, NPTS // 16], BF16)
            nc.sync.dma_start(out=iyw[:], in_=iyw_in.ap())
            pjx = cpool.tile([128, NGRP], BF16)
            nc.sync.dma_start(out=pjx[:], in_=pjx_in.ap())
            pjy = cpool.tile([128, NGRP], BF16)
            nc.sync.dma_start(out=pjy[:], in_=pjy_in.ap())

            ztail = cpool.tile([1, WINEL], BF16)
            nc.vector.memset(ztail[:], 0.0)
            for _gy in gys:
                nc.sync.dma_start(
                    out=AP(_gy.ap().tensor, GYSZ - WINEL, [[1, WINEL]]),
                    in_=ztail[:],
                )

            V = nc.vector
            S = nc.scalar

            for gi in range(gbufs):
                gz = gpool.tile([128, gcall * JCHUNK, WINEL], BF16, tag="g",
                                name=f"gz{gi}")
                nc.vector.memset(gz[:], 0.0)

            def tsc(out, in0, s1, op0, s2=None, op1=None):
                if s2 is None:
                    return V.tensor_scalar(out, in0, s1, None, op0)
                return V.tensor_scalar(out, in0, s1, s2, op0, op1)

            A = mybir.AluOpType

            def batch_phases(b):
                xb = x_in.ap()[b]  # [64, HW] f32
                ob = out_ext.ap()[b]
                gy = gys[b].ap()  # [GYSZ] bf16

                # ---------- Phase A: build GY ----------
                # X load slices: [128=(par,ch), 16 rowpairs * 256 x] bf16.
                # partition p = par*64 + c holds row 2k+par of channel c.
                # src el X[c, (2k+par)*256 + x]
                def load_slice(k0):
                    xt = xpool.tile([128, 8 * 256], BF16, tag="xs")
                    for par in range(2):
                        src = AP(
                            xb.tensor,
                            xb.offset + k0 * 4096 + par * 256,
                            [[HW, C], [512, 8], [1, 256]],
                        )
                        nc.gpsimd.dma_start(
                            out=xt[par * 64 : (par + 1) * 64, :], in_=src
                        )
                    return xt

                # hs quarter tiles: [128, 2 xh, NQH+1 slots, 128] bf16
                _hqn = [0]
                def new_hq():
                    _hqn[0] += 1
                    return hpool.tile([128, 2, NQH + 1, 128], BF16, tag="hq",
                                      name=f"hq{b}_{_hqn[0]}")

                hq = new_hq()
                xt = load_slice(0)
                xt_next = load_slice(1)
                for P in range(128):
                    if P > 0 and P % 8 == 0:
                        xt = xt_next
                        xt_next = (
                            load_slice(P // 8 + 1) if P // 8 + 1 < 16 else None
                        )
                    cur = xt  # xt holds slice P//8
                    ps = apsum.tile([128, 256], BF16, tag="psA")
                    base = (P % 8) * 256
                    for xh in range(2):
                        nc.tensor.transpose(
                            ps[:, xh * 128 : (xh + 1) * 128],
                            cur[:, base + xh * 128 : base + xh * 128 + 128],
                            identb[:],
                        )
                    # copy ps -> hq slot (P - h0); boundary P lands in 2 tiles
                    q, slot = divmod(P, NQH)
                    psv = ps[:].rearrange("p (xh e) -> p xh e", xh=2)
                    dst = AP(
                        hq[:].tensor,
                        hq[:].offset + slot * 128,
                        [hq[:].ap[0], [(NQH + 1) * 128, 2], [1, 128]],
                    )
                    S.copy(dst, psv)
                    if slot == NQH - 1 or P == 127:
                        # before flushing, need slot NQH content = next P's data
                        # (record h0+NQH-1 spans P=h0+NQH-1, h0+NQH). For
                        # P=127 the next rows (256,257) are zero.
                        if P == 127:
                            zdst = AP(
                                hq[:].tensor,
                                hq[:].offset + NQH * 128,
                                [hq[:].ap[0], [(NQH + 1) * 128, 2], [1, 128]],
                            )
                            V.memset(zdst, 0.0)
                        else:
                            # next P's transpose: compute now into this tile
                            # AND it will also be copied into next tile's
                            # slot 0 on the next loop iteration.
                            P2 = P + 1
                            if P2 % 8 == 0:
                                nxt = xt_next
                            else:
                                nxt = xt
                            ps2 = apsum.tile([128, 256], BF16, tag="psA")
                            b2 = (P2 % 8) * 256
                            for xh in range(2):
                                nc.tensor.transpose(
                                    ps2[:, xh * 128 : (xh + 1) * 128],
                                    nxt[:, b2 + xh * 128 : b2 + xh * 128 + 128],
                                    identb[:],
                                )
                            zdst = AP(
                                hq[:].tensor,
                                hq[:].offset + NQH * 128,
                                [hq[:].ap[0], [(NQH + 1) * 128, 2], [1, 128]],
                            )
                            S.copy(zdst, ps2[:].rearrange("p (xh e) -> p xh e", xh=2))
                        # flush records h in [q*NQH, q*NQH+NQH), per xh
                        h0 = q * NQH
                        for xh in range(2):
                            dstg = AP(
                                gy.tensor,
                                h0 * 65536 + xh * 32768,
                                [[256, 128], [65536, NQH], [1, 256]],
                            )
                            srcg = AP(
                                hq[:].tensor,
                                hq[:].offset + xh * (NQH + 1) * 128,
                                [hq[:].ap[0], [128, NQH], [1, 256]],
                            )
                            nc.sync.dma_start(out=dstg, in_=srcg)
                        if P < 127:
                            hq = new_hq()

                yield "A"
                if "C" not in phases:
                    return
                # ---------- Phase B: theta-derived scalars ----------
                thsb = spool.tile([128, 6], F32, tag="thsb")
                nc.sync.dma_start(out=thsb[:], in_=th_in.ap()[b])
                thb = thsb
                sc = spool.tile([128, 8], F32, tag="thsc")
                tsc(sc[:, 0:1], thb[:, 0:1], SCALE, A.mult)
                tsc(sc[:, 1:2], thb[:, 1:2], SCALE, A.mult)
                V.tensor_tensor(sc[:, 2:3], thb[:, 2:3], thb[:, 0:1], A.subtract)
                V.tensor_tensor(sc[:, 2:3], sc[:, 2:3], thb[:, 1:2], A.subtract)
                tsc(sc[:, 2:3], sc[:, 2:3], 1.0, A.add, float(W // 2), A.mult)
                tsc(sc[:, 3:4], thb[:, 3:4], SCALE, A.mult)
                tsc(sc[:, 4:5], thb[:, 4:5], SCALE, A.mult)
                V.tensor_tensor(sc[:, 5:6], thb[:, 5:6], thb[:, 3:4], A.subtract)
                V.tensor_tensor(sc[:, 5:6], sc[:, 5:6], thb[:, 4:5], A.subtract)
                tsc(sc[:, 5:6], sc[:, 5:6], 1.0, A.add, float(H // 2), A.mult)
                ax, bx, cx = sc[:, 0:1], sc[:, 1:2], sc[:, 2:3]
                ay, by, cy = sc[:, 3:4], sc[:, 4:5], sc[:, 5:6]

                # ---------- Phase C: gather indices [128, 4096] i16 ----------
                flagsb = wpool.tile([128, NCHUNK], I32, tag="flagsb")
                nc.sync.dma_start(out=flagsb[:], in_=chf_in.ap()[b])
                cntsb = wpool.tile([1, NCHUNK], I32, tag="cntsb")
                nc.sync.dma_start(out=cntsb[:], in_=chc_in.ap()[b])
                SW = NPTS // 16
                NSPL = 8
                SH = SW // NSPL
                idx16 = wpool.tile([128, SW], I16, tag="idx16")
                for hh in range(NSPL):
                    hsl = slice(hh * SH, (hh + 1) * SH)
                    t0 = spool.tile([128, SH], F32, tag="wk0")
                    t1 = spool.tile([128, SH], F32, tag="wk1")
                    i0 = spool.tile([128, SH], I32, tag="wki0")
                    i1 = spool.tile([128, SH], I32, tag="wki1")
                    # x0c = floor(clamp(x, 0, 254))
                    tsc(t0[:], ixw[:, hsl], ax, A.mult)
                    V.scalar_tensor_tensor(t0[:], iyw[:, hsl], bx, t0[:], A.mult, A.add)
                    tsc(t0[:], t0[:], cx, A.add, 0.0, A.max)
                    tsc(t0[:], t0[:], float(W - 2), A.min, 0.5, A.subtract)
                    V.tensor_copy(i0[:], t0[:])  # RNE -> floor
                    # y0c = floor(clamp(y, 0, 254)); idx = (y0c>>1)<<8 | x0c
                    tsc(t1[:], ixw[:, hsl], ay, A.mult)
                    V.scalar_tensor_tensor(t1[:], iyw[:, hsl], by, t1[:], A.mult, A.add)
                    tsc(t1[:], t1[:], cy, A.add, 0.0, A.max)
                    tsc(t1[:], t1[:], float(H - 2), A.min, 0.5, A.subtract)
                    V.tensor_copy(i1[:], t1[:])
                    tsc(i1[:], i1[:], 1, A.arith_shift_right)
                    tsc(i1[:], i1[:], 8, A.logical_shift_left)
                    V.tensor_tensor(i1[:], i1[:], i0[:], A.add)
                    # mask fully-invalid chunks to -1 (idx = (idx+1)*flag - 1)
                    # but keep each chunk's first s-column (16 idxs) valid so
                    # the gather ucode always has >=16 leading non-negatives.
                    ch0 = hh * (NCHUNK // NSPL)
                    chn = NCHUNK // NSPL
                    i1v = i1[:].rearrange("p (ch r) -> p ch r", ch=chn)
                    fl = (
                        flagsb[:, ch0 : ch0 + chn].unsqueeze(2)
                        .to_broadcast([128, chn, 127])
                    )
                    tsc(i1[:], i1[:], 1, A.add)
                    V.tensor_tensor(i1v[:, :, 1:], i1v[:, :, 1:], fl, A.mult)
                    tsc(i1[:], i1[:], 1, A.subtract)
                    V.tensor_copy(idx16[:, hsl], i1[:])

                # ---------- Phase D: 6-slot weights [128, NGRP*6] ----------
                W8b = wpool.tile([128, NGRP * 6], BF16, tag="W8b")
                W8v = W8b[:].rearrange("p (j s) -> p j s", s=6)

                xv = spool.tile([128, NGRP], F32, tag="xv")
                yv = spool.tile([128, NGRP], F32, tag="yv")
                u0 = spool.tile([128, NGRP], F32, tag="u0")
                u1 = spool.tile([128, NGRP], F32, tag="u1")
                u2 = spool.tile([128, NGRP], F32, tag="u2")
                u3 = spool.tile([128, NGRP], F32, tag="u3")
                iw0 = spool.tile([128, NGRP], I32, tag="iw0")
                tsc(xv[:], pjx[:], ax, A.mult)
                V.scalar_tensor_tensor(xv[:], pjy[:], bx, xv[:], A.mult, A.add)
                tsc(xv[:], xv[:], cx, A.add)
                tsc(yv[:], pjx[:], ay, A.mult)
                V.scalar_tensor_tensor(yv[:], pjy[:], by, yv[:], A.mult, A.add)
                tsc(yv[:], yv[:], cy, A.add)
                # valid mask -> u0
                tsc(u0[:], xv[:], 0.0, A.is_ge)
                tsc(u1[:], xv[:], float(W - 1), A.is_lt)
                V.tensor_tensor(u0[:], u0[:], u1[:], A.mult)
                tsc(u1[:], yv[:], 0.0, A.is_ge)
                V.tensor_tensor(u0[:], u0[:], u1[:], A.mult)
                tsc(u1[:], yv[:], float(H - 1), A.is_lt)
                V.tensor_tensor(u0[:], u0[:], u1[:], A.mult)
                # x side: u1 = x+ ; u2 = fx
                tsc(u1[:], xv[:], 0.0, A.max)
                tsc(u2[:], u1[:], 0.5, A.subtract)
                V.tensor_copy(iw0[:], u2[:])
                V.tensor_copy(u2[:], iw0[:])
                V.tensor_tensor(u2[:], u1[:], u2[:], A.subtract)  # fx
                # wxv0 = (1-fx)*valid -> xv ; wxv1 = fx*valid -> u2
                tsc(xv[:], u2[:], -1.0, A.mult, 1.0, A.add)
                V.tensor_tensor(xv[:], xv[:], u0[:], A.mult)
                V.tensor_tensor(u2[:], u2[:], u0[:], A.mult)
                # y side: u1 = y+ ; yv = fy ; u3 = pary
                tsc(u1[:], yv[:], 0.0, A.max)
                tsc(yv[:], u1[:], 0.5, A.subtract)
                V.tensor_copy(iw0[:], yv[:])
                V.tensor_copy(yv[:], iw0[:])
                V.tensor_tensor(yv[:], u1[:], yv[:], A.subtract)  # fy
                tsc(iw0[:], iw0[:], 1, A.bitwise_and)
                V.tensor_copy(u3[:], iw0[:])  # pary
                # u1 = 1-pary (parc), u0 free after folding into wx
                parc = u1
                tsc(parc[:], u3[:], -1.0, A.mult, 1.0, A.add)
                wy0 = u0  # reuse: wy0 = 1-fy (valid already folded into wx)
                tsc(wy0[:], yv[:], -1.0, A.mult, 1.0, A.add)
                # wys0 = wy0*parc; wys1 = wy0*pary + fy*parc; wys2 = fy*pary
                wys0 = spool.tile([128, NGRP], F32, tag="wys0")
                wys1 = spool.tile([128, NGRP], F32, tag="wys1")
                wys2 = spool.tile([128, NGRP], F32, tag="wys2")
                V.tensor_tensor(wys0[:], wy0[:], parc[:], A.mult)
                V.tensor_tensor(wys1[:], wy0[:], u3[:], A.mult)
                V.tensor_tensor(parc[:], yv[:], parc[:], A.mult)
                V.tensor_tensor(wys1[:], wys1[:], parc[:], A.add)
                V.tensor_tensor(wys2[:], yv[:], u3[:], A.mult)
                # W6 slots: dx*3 + yi
                V.tensor_tensor(W8v[:, :, 0], xv[:], wys0[:], A.mult)
                V.tensor_tensor(W8v[:, :, 1], xv[:], wys1[:], A.mult)
                V.tensor_tensor(W8v[:, :, 2], xv[:], wys2[:], A.mult)
                V.tensor_tensor(W8v[:, :, 3], u2[:], wys0[:], A.mult)
                V.tensor_tensor(W8v[:, :, 4], u2[:], wys1[:], A.mult)
                V.tensor_tensor(W8v[:, :, 5], u2[:], wys2[:], A.mult)
                W8bv = W8v

                yield "CD"
                if "E" not in phases:
                    return
                # ---------- Phase E: gather + combine + transpose + out ----
                in_gy = AP(gy.tensor, 0, [[RECEL, NREC], [1, WINEL]])
                cpts = jchunk * 128
                nch = (NGRP // jchunk) if n_chunks == NCHUNK else n_chunks
                gcpts = cpts * gcall
                if "creg" not in _regcache:
                    _regcache["creg"] = nc.gpsimd.alloc_register("creg")
                creg = _regcache["creg"]
                gbig = None
                for ci in range(nch):
                    if ci % gcall == 0 and not (nogather and gbig is not None):
                        gbig = gpool.tile(
                            [128, gcall * jchunk, WINEL], BF16, tag="g",
                            name=f"g{b}_{ci}",
                        )
                        gidxs = idx16[
                            :, ci * (cpts // 16) : (ci + gcall) * (cpts // 16)
                        ]
                        nc.gpsimd.reg_load(creg, cntsb[0:1, ci : ci + 1])
                        nc.gpsimd.dma_gather(
                            gbig[:], in_gy, gidxs, gcpts, creg, WINEL,
                            elem_step=RECEL, queue_num=(ci // gcall) % nq,
                            single_packet=sp,
                        )
                    if nocombine:
                        continue
                    sub = ci % gcall
                    g = gbig[:, sub * jchunk : (sub + 1) * jchunk, :]
                    gv = g.rearrange("p j (s c) -> p j s c", c=64)
                    cj = slice(ci * jchunk, (ci + 1) * jchunk)
                    w3a = (
                        W8bv[:, cj, 0:3].unsqueeze(3)
                        .to_broadcast([128, jchunk, 3, 64])
                    )
                    w3b = (
                        W8bv[:, cj, 3:6].unsqueeze(3)
                        .to_broadcast([128, jchunk, 3, 64])
                    )
                    m1 = mpool.tile([128, jchunk, 3, 64], BF16, tag="m1")
                    m2 = mpool.tile([128, jchunk, 3, 64], BF16, tag="m2")
                    V.tensor_tensor(m1[:], gv[:, :, 0:3, :], w3a, A.mult)
                    V.tensor_tensor(m2[:], gv[:, :, 4:7, :], w3b, A.mult)
                    V.tensor_tensor(m1[:], m1[:], m2[:], A.add)
                    comb = opool.tile([128, jchunk, 64], BF16, tag="comb")
                    V.tensor_tensor(comb[:], m1[:, :, 0, :], m1[:, :, 1, :], A.add)
                    V.tensor_tensor(comb[:], comb[:], m1[:, :, 2, :], A.add)

                    # transpose [pt, c] -> [c, pt]: J/2 transposes of [128,128]
                    ps = opsum.tile([128, (jchunk // 2) * 128], BF16, tag="psO")
                    combv = comb[:].rearrange("p j c -> p (j c)")
                    for t in range(jchunk // 2):
                        nc.tensor.transpose(
                            ps[:, t * 128 : (t + 1) * 128],
                            combv[:, t * 128 : (t + 1) * 128],
                            identb[:],
                        )
                    # psum [(j&1)*64+c, (j>>1)*128+pt] -> outsb [c, j*128+pt]
                    outsb = opool.tile([C, jchunk * 128], F32, tag="outsb")
                    psv = ps[:].rearrange("p (t q) -> p t q", q=128)
                    osv = outsb[:].rearrange("c (j q) -> c j q", q=128)
                    S.copy(osv[:, 0::2, :], psv[0:64, :, :])
                    S.copy(osv[:, 1::2, :], psv[64:128, :, :])
                    iy0 = (ci // TGRID_X) * TILE_IY
                    ix0 = (ci % TGRID_X) * TILE_IX
                    dsto = AP(
                        ob.tensor,
                        ob.offset + iy0 * W + ix0,
                        [[HW, C], [W, TILE_IY], [1, TILE_IX]],
                    )
                    nc.sync.dma_start(out=dsto, in_=outsb[:])
                yield "E"

            for _rep in range(reps):
                gens = [batch_phases(b) for b in range(n_batches)]
                if order == "interleave" and n_batches == 2:
                    g0, g1 = gens
                    next(g0)        # A0
                    next(g0)        # CD0
                    next(g1)        # A1
                    for _ in g0:    # E0
                        pass
                    for _ in g1:    # CD1, E1
                        pass
                else:
                    for g_ in gens:
                        for _ in g_:
                            pass

    nc.compile()
    return nc


_CONSTS = _host_consts()


def _batch_perm(flags):
    """Pair heavy with light batches: perm[2k], perm[2k+1] -> core k."""
    loads = flags.sum(axis=1)
    order = np.argsort(-loads, kind="stable")
    perm = []
    for k in range(NCORES):
        perm.append(int(order[k]))
        perm.append(int(order[2 * NCORES - 1 - k]))
    return perm


def _make_in_maps(X, theta, n_batches=BPC, perm=None):
    Xr = np.ascontiguousarray(X.reshape(B, C, HW), np.float32)
    th = np.ascontiguousarray(theta, np.float32)
    flags = _host_chunk_flags(th)  # [B, NCHUNK] int32
    cnts = np.where(flags > 0, CHUNK_PTS, 16).astype(np.int32)
    if perm is None:
        perm = list(range(B))
    in_maps = []
    for core in range(NCORES):
        bsel = perm[core * n_batches : (core + 1) * n_batches]
        th_rep = np.repeat(th[bsel][:, None, :], 128, axis=1)
        chf = np.repeat(flags[bsel][:, None, :], 128, axis=1)
        in_maps.append(
            {
                "X": np.ascontiguousarray(Xr[bsel]),
                "THETA": np.ascontiguousarray(th_rep, np.float32),
                "CHFLAG": np.ascontiguousarray(chf, np.int32),
                "CHCNT": np.ascontiguousarray(cnts[bsel][:, None, :], np.int32),
                **_CONSTS,
            }
        )
    return in_maps


_NC_CACHE = {}


def kernel(X, affine_transformation):
    from concourse.bass_utils import run_bass_kernel_spmd

    X = np.asarray(X, np.float32)
    theta = np.asarray(affine_transformation, np.float32)
    if "nc" not in _NC_CACHE:
        _NC_CACHE["nc"] = build_nc()
    nc = _NC_CACHE["nc"]
    flags = _host_chunk_flags(theta)
    perm = _batch_perm(flags)
    in_maps = _make_in_maps(X, theta, perm=perm)
    res = run_bass_kernel_spmd(nc, in_maps, list(range(NCORES)))
    outs = np.concatenate(
        [r["OUT"].reshape(BPC, C, H, W) for r in res.results], axis=0
    )
    full = np.empty_like(outs)
    full[perm] = outs
    return full


if __name__ == "__main__":
    mode = sys.argv[1] if len(sys.argv) > 1 else "sim"
    if mode == "build":
        nc = build_nc()
        print("build ok")
    elif mode == "sim":
        n_chunks = int(sys.argv[2]) if len(sys.argv) > 2 else 2
        import concourse.bass_interp as bass_interp

        _orig_copy = bass_interp.InstructionExecutor.visit_InstTensorCopy

        def _copy_rne(self, instruction, *, reg_snapshot=None):
            from concourse.bass_interp import Direction, InterpAPClass

            inp, outp = instruction.ins[0], instruction.outs[0]
            if isinstance(inp, InterpAPClass) and isinstance(outp, InterpAPClass):
                iv = self.view_ap(
                    inp, Direction.READ, instruction, reg_snapshot=reg_snapshot
                )
                ov = self.view_ap(
                    outp, Direction.WRITE, instruction, reg_snapshot=reg_snapshot
                )
                if np.issubdtype(iv.dtype, np.floating) and np.issubdtype(
                    ov.dtype, np.integer
                ):
                    ov[:] = np.round(iv.reshape(ov.shape))
                    return
            return _orig_copy(self, instruction, reg_snapshot=reg_snapshot)

        bass_interp.InstructionExecutor.visit_InstTensorCopy = _copy_rne

        rng = np.random.default_rng(0)
        Xt = rng.standard_normal((1, C, HW), dtype=np.float32)
        th = rng.standard_normal((1, 6), dtype=np.float32) * 0.7
        nc = build_nc(n_batches=1, n_chunks=n_chunks, debug=False, order="seq")
        th_rep = np.repeat(th[:, None, :], 128, axis=1)
        sim = bass_interp.CoreSim(nc)
        sim.tensor("X")[:] = Xt
        sim.tensor("THETA")[:] = np.ascontiguousarray(th_rep, np.float32)
        flags = _host_chunk_flags(th)
        cnts = np.where(flags > 0, CHUNK_PTS, 16).astype(np.int32)
        sim.tensor("CHFLAG")[:] = np.repeat(flags[:, None, :], 128, axis=1)
        sim.tensor("CHCNT")[:] = cnts[:, None, :]
        for k, v in _CONSTS.items():
            sim.tensor(k)[:] = v
        sim.simulate()
        got = np.array(sim.tensor("OUT"))

        def ref(Xf, thf):
            xl = np.linspace(-1, 1, W, dtype=np.float32)
            yl = np.linspace(-1, 1, H, dtype=np.float32)
            xc, yc = np.meshgrid(xl, yl, indexing="ij")
            grid = np.stack([xc.ravel(), yc.ravel(), np.ones(W * H, np.float32)], 0)
            thr = thf.reshape(-1, 2, 3)
            sampled = np.einsum("bij,jn->bin", thr, grid)
            x = (sampled[:, 0, :] + 1) * (W * 0.5)
            y = (sampled[:, 1, :] + 1) * (H * 0.5)
            x0 = np.clip(np.floor(x).astype(np.int64), 0, W - 1)
            x1 = np.clip(np.floor(x).astype(np.int64) + 1, 0, W - 1)
            y0 = np.clip(np.floor(y).astype(np.int64), 0, H - 1)
            y1 = np.clip(np.floor(y).astype(np.int64) + 1, 0, H - 1)
            flat = Xf.reshape(-1, C, H * W).transpose(0, 2, 1)
            bidx = np.arange(flat.shape[0])[:, None]
            pa = flat[bidx, y0 * W + x0]
            pb = flat[bidx, y1 * W + x0]
            pc = flat[bidx, y0 * W + x1]
            pd = flat[bidx, y1 * W + x1]
            x0f, x1f, y0f, y1f = (a.astype(np.float32) for a in (x0, x1, y0, y1))
            wa = ((x1f - x) * (y1f - y))[..., None]
            wb = ((x1f - x) * (y - y0f))[..., None]
            wc = ((x - x0f) * (y1f - y))[..., None]
            wd = ((x - x0f) * (y - y0f))[..., None]
            out = wa * pa + wb * pb + wc * pc + wd * pd
            return out.reshape(-1, W, H, C).transpose(0, 3, 2, 1)

        exp_full = ref(Xt, th).reshape(1, C, H, W)
        gotr = got[0].reshape(C, H, W)
        print("chunk flags:", flags[0][:n_chunks], "...")
        gs, es = [], []
        for ci in range(n_chunks):
            iy0 = (ci // TGRID_X) * TILE_IY
            ix0 = (ci % TGRID_X) * TILE_IX
            gs.append(gotr[:, iy0 : iy0 + TILE_IY, ix0 : ix0 + TILE_IX].ravel())
            es.append(
                exp_full[0][:, iy0 : iy0 + TILE_IY, ix0 : ix0 + TILE_IX].ravel()
            )
        got_s = np.concatenate(gs)
        exp_s = np.concatenate(es)
        err = np.abs(got_s - exp_s)
        denom = np.abs(exp_s).max() + 1e-8
        print("max abs err:", err.max(), " max |exp|:", np.abs(exp_s).max())
        print(
            "rel l2:",
            np.linalg.norm(got_s - exp_s) / (np.linalg.norm(exp_s) + 1e-8),
        )
        bad = np.argwhere(err > 3e-2 * denom)
        print("n bad:", len(bad), "of", got_s.size)
        if len(bad):
            print("first bad:", bad[:5])


# revision 3
# speedup vs baseline: 1.0653x; 1.0653x over previous
"""Bilinear interpolation (affine grid sample) TRN2 Bass kernel, v5.

v5 = v4 + data-dependent chunk skipping:
  - Points are enumerated in 32(iy) x 64(ix) spatial tiles (32 chunks of
    2048/batch).  Host computes per-chunk "any maybe-valid point" flags
    from theta (conservative eps=0.01 margin); fully-out-of-bounds chunks
    gather ZERO windows (num_idxs_reg loaded from an input tensor, idx=-1)
    while the combine still runs (weights=0 -> zero output).
  - Host pairs heavy+light batches per core to balance skipped work.

Changes vs v2 (kernel.py):
  - Phase A rebuilt: X loaded as [128=(rowparity,ch), k*256+x] bf16 slices;
    256 full-width [128,128] PE transposes per batch (vs 512 half-width);
    ONE ACT copy per row-pair P (vs 2); hs quarter tiles flushed by big
    strided DMA.
  - Phase E combine trimmed to 6 real slots (skip zero slots 3,7):
    3072*3 + 1024*2 DVE els/chunk vs 4096*3+2048+1024.
  - Program order A0 CD0 A1 E0 CD1 E1 so batch-1 build overlaps batch-0
    gather/combine; idx/weight pools double-buffered.
"""

import sys

sys.path.insert(0, "/opt/trn_rl_repo")

import numpy as np
import ml_dtypes  # noqa: F401

import concourse.bass as bass
import concourse.bacc as bacc
import concourse.mybir as mybir
from concourse import tile
from concourse.bass import AP
from concourse.masks import make_identity

F32 = mybir.dt.float32
BF16 = mybir.dt.bfloat16
I16 = mybir.dt.int16
I32 = mybir.dt.int32

B, C, H, W = 16, 64, 256, 256
HW = H * W
NCORES = 8
BPC = B // NCORES  # 2

NPTS = HW
NGRP = NPTS // 128  # 512
JCHUNK = 16
NCHUNK = NGRP // JCHUNK
CHUNK_PTS = JCHUNK * 128

NREC = 32768  # records of 256 bf16 el (512 B); u = (y0>>1)*256 + x0
RECEL = 256  # elements per record step
WINEL = 512  # gather window: 2 records = 8 px = 1 KB
GYSZ = 128 * 65536 + WINEL  # + pad so the u=32767 AP row stays in-bounds

SCALE = 2.0 * (W / 2) / (W - 1)  # 256/255

NQH = 16  # h-records per hs slice tile


TILE_IY, TILE_IX = 32, 64  # chunk = one spatial tile
TGRID_X = W // TILE_IX  # 4


def _i_to_ixiy(i):
    """gather/combine position i -> (ix, iy) under tile enumeration."""
    i = np.asarray(i)
    ci, local = i // CHUNK_PTS, i % CHUNK_PTS
    bi, bj = ci // TGRID_X, ci % TGRID_X
    iyl, ixl = local // TILE_IX, local % TILE_IX
    return bj * TILE_IX + ixl, bi * TILE_IY + iyl


def _host_consts():
    q = np.arange(128)
    s = np.arange(NPTS // 16)
    j = np.arange(NGRP)
    p = np.arange(128)
    # wrapped idx layout: gather position i = 16*s + (q%16)
    iw = 16 * s[None, :] + (q % 16)[:, None]
    ixw, iyw = _i_to_ixiy(iw)
    # weight layout: position i = j*128 + p
    pw = 128 * j[None, :] + p[:, None]
    pjx, pjy = _i_to_ixiy(pw)
    return {
        "IXW": np.ascontiguousarray(ixw.astype(np.float64), "bfloat16"),
        "IYW": np.ascontiguousarray(iyw.astype(np.float64), "bfloat16"),
        "PJX": np.ascontiguousarray(pjx.astype(np.float64), "bfloat16"),
        "PJY": np.ascontiguousarray(pjy.astype(np.float64), "bfloat16"),
    }


def _host_chunk_flags(theta):
    """Conservative per-chunk any-maybe-valid flags [B, NCHUNK] from theta."""
    th = np.asarray(theta, np.float64)
    Bn = th.shape[0]
    ixg = np.arange(W)
    iyg = np.arange(H)
    IXg, IYg = np.meshgrid(ixg, iyg, indexing="xy")  # [iy, ix]
    Sg = 2.0 / (W - 1)
    eps = 0.01
    flags = np.zeros((Bn, NCHUNK), np.int32)
    for b in range(Bn):
        t = th[b].reshape(2, 3)
        xc = IXg * Sg - 1.0
        yc = IYg * Sg - 1.0
        xs = (t[0, 0] * xc + t[0, 1] * yc + t[0, 2] + 1) * (W / 2)
        ys = (t[1, 0] * xc + t[1, 1] * yc + t[1, 2] + 1) * (H / 2)
        v = (xs > -eps) & (xs < W - 1 + eps) & (ys > -eps) & (ys < H - 1 + eps)
        tiles = v.reshape(H // TILE_IY, TILE_IY, W // TILE_IX, TILE_IX)
        anyv = tiles.any(axis=(1, 3))  # [bands, cols]
        flags[b] = anyv.reshape(-1).astype(np.int32)
    return flags


def build_nc(n_batches=BPC, n_chunks=NCHUNK, debug=False, jchunk=JCHUNK,
             gbufs=2, sp=False, gcall=1, nq=2, scratch=49152,
             phases="ACDE", nogather=False, nocombine=False, order="interleave",
             reps=1):
    nc = bacc.Bacc("TRN2", target_bir_lowering=False, debug=debug,
                   num_swdge_queues=nq, dynamic_dma_scratch_size=scratch)

    x_in = nc.declare_dram_parameter("X", [n_batches, C, HW], F32, isOutput=False)
    th_in = nc.declare_dram_parameter("THETA", [n_batches, 128, 6], F32, isOutput=False)
    ixw_in = nc.declare_dram_parameter("IXW", [128, NPTS // 16], BF16, isOutput=False)
    iyw_in = nc.declare_dram_parameter("IYW", [128, NPTS // 16], BF16, isOutput=False)
    pjx_in = nc.declare_dram_parameter("PJX", [128, NGRP], BF16, isOutput=False)
    pjy_in = nc.declare_dram_parameter("PJY", [128, NGRP], BF16, isOutput=False)
    chf_in = nc.declare_dram_parameter("CHFLAG", [n_batches, 128, NCHUNK], I32,
                                       isOutput=False)
    chc_in = nc.declare_dram_parameter("CHCNT", [n_batches, 1, NCHUNK], I32,
                                       isOutput=False)
    out_ext = nc.declare_dram_parameter("OUT", [n_batches, C, HW], F32, isOutput=True)

    gys = [nc.dram_tensor(f"gy{b}", [GYSZ], BF16) for b in range(n_batches)]

    _regcache = {}

    with tile.TileContext(nc) as tc:
        import contextlib

        with contextlib.ExitStack() as ctx:
            cpool = ctx.enter_context(tc.tile_pool(name="consts", bufs=1))
            xpool = ctx.enter_context(tc.tile_pool(name="xload", bufs=2))
            apsum = ctx.enter_context(tc.tile_pool(name="tpsum", bufs=2, space="PSUM"))
            hpool = ctx.enter_context(tc.tile_pool(name="hsbuf", bufs=2))
            wpool = ctx.enter_context(tc.tile_pool(name="weights", bufs=2))
            spool = ctx.enter_context(tc.tile_pool(name="scratch", bufs=1))
            gpool = ctx.enter_context(tc.tile_pool(name="gather", bufs=gbufs))
            mpool = ctx.enter_context(tc.tile_pool(name="mulbuf", bufs=1))
            opool = ctx.enter_context(tc.tile_pool(name="outbuf", bufs=2))
            opsum = ctx.enter_context(tc.tile_pool(name="opsum", bufs=2, space="PSUM"))

            identb = cpool.tile([128, 128], BF16)
            make_identity(nc, identb[:])
            ixw = cpool.tile([128, NPTS // 16], BF16)
            nc.sync.dma_start(out=ixw[:], in_=ixw_in.ap())
            iyw = cpool.tile([128, NPTS // 16], BF16)
            nc.sync.dma_start(out=iyw[:], in_=iyw_in.ap())
            pjx = cpool.tile([128, NGRP], BF16)
            nc.sync.dma_start(out=pjx[:], in_=pjx_in.ap())
            pjy = cpool.tile([128, NGRP], BF16)
            nc.sync.dma_start(out=pjy[:], in_=pjy_in.ap())

            ztail = cpool.tile([1, WINEL], BF16)
            nc.vector.memset(ztail[:], 0.0)
            for _gy in gys:
                nc.sync.dma_start(
                    out=AP(_gy.ap().tensor, GYSZ - WINEL, [[1, WINEL]]),
                    in_=ztail[:],
                )

            V = nc.vector
            S = nc.scalar

            for gi in range(gbufs):
                gz = gpool.tile([128, gcall * JCHUNK, WINEL], BF16, tag="g",
                                name=f"gz{gi}")
                nc.vector.memset(gz[:], 0.0)

            def tsc(out, in0, s1, op0, s2=None, op1=None):
                if s2 is None:
                    return V.tensor_scalar(out, in0, s1, None, op0)
                return V.tensor_scalar(out, in0, s1, s2, op0, op1)

            A = mybir.AluOpType

            def batch_phases(b):
                xb = x_in.ap()[b]  # [64, HW] f32
                ob = out_ext.ap()[b]
                gy = gys[b].ap()  # [GYSZ] bf16

                # ---------- Phase A: build GY ----------
                # X load slices: [128=(par,ch), 16 rowpairs * 256 x] bf16.
                # partition p = par*64 + c holds row 2k+par of channel c.
                # src el X[c, (2k+par)*256 + x]
                def load_slice(k0):
                    xt = xpool.tile([128, 8 * 256], BF16, tag="xs")
                    for par in range(2):
                        src = AP(
                            xb.tensor,
                            xb.offset + k0 * 4096 + par * 256,
                            [[HW, C], [512, 8], [1, 256]],
                        )
                        nc.gpsimd.dma_start(
                            out=xt[par * 64 : (par + 1) * 64, :], in_=src
                        )
                    return xt

                # hs quarter tiles: [128, 2 xh, NQH+1 slots, 128] bf16
                _hqn = [0]
                def new_hq():
                    _hqn[0] += 1
                    return hpool.tile([128, 2, NQH + 1, 128], BF16, tag="hq",
                                      name=f"hq{b}_{_hqn[0]}")

                hq = new_hq()
                xt = load_slice(0)
                xt_next = load_slice(1)
                for P in range(128):
                    if P > 0 and P % 8 == 0:
                        xt = xt_next
                        xt_next = (
                            load_slice(P // 8 + 1) if P // 8 + 1 < 16 else None
                        )
                    cur = xt  # xt holds slice P//8
                    ps = apsum.tile([128, 256], BF16, tag="psA")
                    base = (P % 8) * 256
                    for xh in range(2):
                        nc.tensor.transpose(
                            ps[:, xh * 128 : (xh + 1) * 128],
                            cur[:, base + xh * 128 : base + xh * 128 + 128],
                            identb[:],
                        )
                    # copy ps -> hq slot (P - h0); boundary P lands in 2 tiles
                    q, slot = divmod(P, NQH)
                    psv = ps[:].rearrange("p (xh e) -> p xh e", xh=2)
                    dst = AP(
                        hq[:].tensor,
                        hq[:].offset + slot * 128,
                        [hq[:].ap[0], [(NQH + 1) * 128, 2], [1, 128]],
                    )
                    S.copy(dst, psv)
                    if slot == NQH - 1 or P == 127:
                        # before flushing, need slot NQH content = next P's data
                        # (record h0+NQH-1 spans P=h0+NQH-1, h0+NQH). For
                        # P=127 the next rows (256,257) are zero.
                        if P == 127:
                            zdst = AP(
                                hq[:].tensor,
                                hq[:].offset + NQH * 128,
                                [hq[:].ap[0], [(NQH + 1) * 128, 2], [1, 128]],
                            )
                            V.memset(zdst, 0.0)
                        else:
                            # next P's transpose: compute now into this tile
                            # AND it will also be copied into next tile's
                            # slot 0 on the next loop iteration.
                            P2 = P + 1
                            if P2 % 8 == 0:
                                nxt = xt_next
                            else:
                                nxt = xt
                            ps2 = apsum.tile([128, 256], BF16, tag="psA")
                            b2 = (P2 % 8) * 256
                            for xh in range(2):
                                nc.tensor.transpose(
                                    ps2[:, xh * 128 : (xh + 1) * 128],
                                    nxt[:, b2 + xh * 128 : b2 + xh * 128 + 128],
                                    identb[:],
                                )
                            zdst = AP(
                                hq[:].tensor,
                                hq[:].offset + NQH * 128,
                                [hq[:].ap[0], [(NQH + 1) * 128, 2], [1, 128]],
                            )
                            S.copy(zdst, ps2[:].rearrange("p (xh e) -> p xh e", xh=2))
                        # flush records h in [q*NQH, q*NQH+NQH), per xh
                        h0 = q * NQH
                        for xh in range(2):
                            dstg = AP(
                                gy.tensor,
                                h0 * 65536 + xh * 32768,
                                [[256, 128], [65536, NQH], [1, 256]],
                            )
                            srcg = AP(
                                hq[:].tensor,
                                hq[:].offset + xh * (NQH + 1) * 128,
                                [hq[:].ap[0], [128, NQH], [1, 256]],
                            )
                            nc.sync.dma_start(out=dstg, in_=srcg)
                        if P < 127:
                            hq = new_hq()

                yield "A"
                if "C" not in phases:
                    return
                # ---------- Phase B: theta-derived scalars ----------
                thsb = spool.tile([128, 6], F32, tag="thsb")
                nc.sync.dma_start(out=thsb[:], in_=th_in.ap()[b])
                thb = thsb
                sc = spool.tile([128, 8], F32, tag="thsc")
                tsc(sc[:, 0:1], thb[:, 0:1], SCALE, A.mult)
                tsc(sc[:, 1:2], thb[:, 1:2], SCALE, A.mult)
                V.tensor_tensor(sc[:, 2:3], thb[:, 2:3], thb[:, 0:1], A.subtract)
                V.tensor_tensor(sc[:, 2:3], sc[:, 2:3], thb[:, 1:2], A.subtract)
                tsc(sc[:, 2:3], sc[:, 2:3], 1.0, A.add, float(W // 2), A.mult)
                tsc(sc[:, 3:4], thb[:, 3:4], SCALE, A.mult)
                tsc(sc[:, 4:5], thb[:, 4:5], SCALE, A.mult)
                V.tensor_tensor(sc[:, 5:6], thb[:, 5:6], thb[:, 3:4], A.subtract)
                V.tensor_tensor(sc[:, 5:6], sc[:, 5:6], thb[:, 4:5], A.subtract)
                tsc(sc[:, 5:6], sc[:, 5:6], 1.0, A.add, float(H // 2), A.mult)
                ax, bx, cx = sc[:, 0:1], sc[:, 1:2], sc[:, 2:3]
                ay, by, cy = sc[:, 3:4], sc[:, 4:5], sc[:, 5:6]

                # ---------- Phase C: gather indices [128, 4096] i16 ----------
                flagsb = wpool.tile([128, NCHUNK], I32, tag="flagsb")
                nc.sync.dma_start(out=flagsb[:], in_=chf_in.ap()[b])
                cntsb = wpool.tile([1, NCHUNK], I32, tag="cntsb")
                nc.sync.dma_start(out=cntsb[:], in_=chc_in.ap()[b])
                SW = NPTS // 16
                NSPL = 8
                SH = SW // NSPL
                idx16 = wpool.tile([128, SW], I16, tag="idx16")
                for hh in range(NSPL):
                    hsl = slice(hh * SH, (hh + 1) * SH)
                    t0 = spool.tile([128, SH], F32, tag="wk0")
                    t1 = spool.tile([128, SH], F32, tag="wk1")
                    i0 = spool.tile([128, SH], I32, tag="wki0")
                    i1 = spool.tile([128, SH], I32, tag="wki1")
                    # x0c = floor(clamp(x, 0, 254))
                    tsc(t0[:], ixw[:, hsl], ax, A.mult)
                    V.scalar_tensor_tensor(t0[:], iyw[:, hsl], bx, t0[:], A.mult, A.add)
                    tsc(t0[:], t0[:], cx, A.add, 0.0, A.max)
                    tsc(t0[:], t0[:], float(W - 2), A.min, 0.5, A.subtract)
                    V.tensor_copy(i0[:], t0[:])  # RNE -> floor
                    # y0c = floor(clamp(y, 0, 254)); idx = (y0c>>1)<<8 | x0c
                    tsc(t1[:], ixw[:, hsl], ay, A.mult)
                    V.scalar_tensor_tensor(t1[:], iyw[:, hsl], by, t1[:], A.mult, A.add)
                    tsc(t1[:], t1[:], cy, A.add, 0.0, A.max)
                    tsc(t1[:], t1[:], float(H - 2), A.min, 0.5, A.subtract)
                    V.tensor_copy(i1[:], t1[:])
                    tsc(i1[:], i1[:], 1, A.arith_shift_right)
                    tsc(i1[:], i1[:], 8, A.logical_shift_left)
                    V.tensor_tensor(i1[:], i1[:], i0[:], A.add)
                    # mask fully-invalid chunks to -1 (idx = (idx+1)*flag - 1)
                    # but keep each chunk's first s-column (16 idxs) valid so
                    # the gather ucode always has >=16 leading non-negatives.
                    ch0 = hh * (NCHUNK // NSPL)
                    chn = NCHUNK // NSPL
                    i1v = i1[:].rearrange("p (ch r) -> p ch r", ch=chn)
                    fl = (
                        flagsb[:, ch0 : ch0 + chn].unsqueeze(2)
                        .to_broadcast([128, chn, 127])
                    )
                    tsc(i1[:], i1[:], 1, A.add)
                    V.tensor_tensor(i1v[:, :, 1:], i1v[:, :, 1:], fl, A.mult)
                    tsc(i1[:], i1[:], 1, A.subtract)
                    V.tensor_copy(idx16[:, hsl], i1[:])

                # ---------- Phase D: 6-slot weights [128, NGRP*6] ----------
                W8b = wpool.tile([128, NGRP * 6], BF16, tag="W8b")
                W8v = W8b[:].rearrange("p (j s) -> p j s", s=6)

                xv = spool.tile([128, NGRP], F32, tag="xv")
                yv = spool.tile([128, NGRP], F32, tag="yv")
                u0 = spool.tile([128, NGRP], F32, tag="u0")
                u1 = spool.tile([128, NGRP], F32, tag="u1")
                u2 = spool.tile([128, NGRP], F32, tag="u2")
                u3 = spool.tile([128, NGRP], F32, tag="u3")
                iw0 = spool.tile([128, NGRP], I32, tag="iw0")
                tsc(xv[:], pjx[:], ax, A.mult)
                V.scalar_tensor_tensor(xv[:], pjy[:], bx, xv[:], A.mult, A.add)
                tsc(xv[:], xv[:], cx, A.add)
                tsc(yv[:], pjx[:], ay, A.mult)
                V.scalar_tensor_tensor(yv[:], pjy[:], by, yv[:], A.mult, A.add)
                tsc(yv[:], yv[:], cy, A.add)
                # valid mask -> u0
                tsc(u0[:], xv[:], 0.0, A.is_ge)
                tsc(u1[:], xv[:], float(W - 1), A.is_lt)
                V.tensor_tensor(u0[:], u0[:], u1[:], A.mult)
                tsc(u1[:], yv[:], 0.0, A.is_ge)
                V.tensor_tensor(u0[:], u0[:], u1[:], A.mult)
                tsc(u1[:], yv[:], float(H - 1), A.is_lt)
                V.tensor_tensor(u0[:], u0[:], u1[:], A.mult)
                # x side: u1 = x+ ; u2 = fx
                tsc(u1[:], xv[:], 0.0, A.max)
                tsc(u2[:], u1[:], 0.5, A.subtract)
                V.tensor_copy(iw0[:], u2[:])
                V.tensor_copy(u2[:], iw0[:])
                V.tensor_tensor(u2[:], u1[:], u2[:], A.subtract)  # fx
                # wxv0 = (1-fx)*valid -> xv ; wxv1 = fx*valid -> u2
                tsc(xv[:], u2[:], -1.0, A.mult, 1.0, A.add)
                V.tensor_tensor(xv[:], xv[:], u0[:], A.mult)
                V.tensor_tensor(u2[:], u2[:], u0[:], A.mult)
                # y side: u1 = y+ ; yv = fy ; u3 = pary
                tsc(u1[:], yv[:], 0.0, A.max)
                tsc(yv[:], u1[:], 0.5, A.subtract)
                V.tensor_copy(iw0[:], yv[:])
                V.tensor_copy(yv[:], iw0[:])
                V.tensor_tensor(yv[:], u1[:], yv[:], A.subtract)  # fy
                tsc(iw0[:], iw0[:], 1, A.bitwise_and)
                V.tensor_copy(u3[:], iw0[:])  # pary
                # u1 = 1-pary (parc), u0 free after folding into wx
                parc = u1
                tsc(parc[:], u3[:], -1.0, A.mult, 1.0, A.add)
                wy0 = u0  # reuse: wy0 = 1-fy (valid already folded into wx)
                tsc(wy0[:], yv[:], -1.0, A.mult, 1.0, A.add)
                # wys0 = wy0*parc; wys1 = wy0*pary + fy*parc; wys2 = fy*pary
                wys0 = spool.tile([128, NGRP], F32, tag="wys0")
                wys1 = spool.tile([128, NGRP], F32, tag="wys1")
                wys2 = spool.tile([128, NGRP], F32, tag="wys2")
                V.tensor_tensor(wys0[:], wy0[:], parc[:], A.mult)
                V.tensor_tensor(wys1[:], wy0[:], u3[:], A.mult)
                V.tensor_tensor(parc[:], yv[:], parc[:], A.mult)
                V.tensor_tensor(wys1[:], wys1[:], parc[:], A.add)
                V.tensor_tensor(wys2[:], yv[:], u3[:], A.mult)
                # W6 slots: dx*3 + yi
                V.tensor_tensor(W8v[:, :, 0], xv[:], wys0[:], A.mult)
                V.tensor_tensor(W8v[:, :, 1], xv[:], wys1[:], A.mult)
                V.tensor_tensor(W8v[:, :, 2], xv[:], wys2[:], A.mult)
                V.tensor_tensor(W8v[:, :, 3], u2[:], wys0[:], A.mult)
                V.tensor_tensor(W8v[:, :, 4], u2[:], wys1[:], A.mult)
                V.tensor_tensor(W8v[:, :, 5], u2[:], wys2[:], A.mult)
                W8bv = W8v

                yield "CD"
                if "E" not in phases:
                    return
                # ---------- Phase E: gather + combine + transpose + out ----
                in_gy = AP(gy.tensor, 0, [[RECEL, NREC], [1, WINEL]])
                cpts = jchunk * 128
                nch = (NGRP // jchunk) if n_chunks == NCHUNK else n_chunks
                gcpts = cpts * gcall
                if "creg" not in _regcache:
                    _regcache["creg"] = nc.gpsimd.alloc_register("creg")
                creg = _regcache["creg"]
                gbig = None
                for ci in range(nch):
                    if ci % gcall == 0 and not (nogather and gbig is not None):
                        gbig = gpool.tile(
                            [128, gcall * jchunk, WINEL], BF16, tag="g",
                            name=f"g{b}_{ci}",
                        )
                        gidxs = idx16[
                            :, ci * (cpts // 16) : (ci + gcall) * (cpts // 16)
                        ]
                        nc.gpsimd.reg_load(creg, cntsb[0:1, ci : ci + 1])
                        nc.gpsimd.dma_gather(
                            gbig[:], in_gy, gidxs, gcpts, creg, WINEL,
                            elem_step=RECEL, queue_num=(ci // gcall) % nq,
                            single_packet=sp,
                        )
                    if nocombine:
                        continue
                    sub = ci % gcall
                    g = gbig[:, sub * jchunk : (sub + 1) * jchunk, :]
                    gv = g.rearrange("p j (s c) -> p j s c", c=64)
                    cj = slice(ci * jchunk, (ci + 1) * jchunk)
                    w3a = (
                        W8bv[:, cj, 0:3].unsqueeze(3)
                        .to_broadcast([128, jchunk, 3, 64])
                    )
                    w3b = (
                        W8bv[:, cj, 3:6].unsqueeze(3)
                        .to_broadcast([128, jchunk, 3, 64])
                    )
                    m1 = mpool.tile([128, jchunk, 3, 64], BF16, tag="m1")
                    m2 = mpool.tile([128, jchunk, 3, 64], BF16, tag="m2")
                    V.tensor_tensor(m1[:], gv[:, :, 0:3, :], w3a, A.mult)
                    V.tensor_tensor(m2[:], gv[:, :, 4:7, :], w3b, A.mult)
                    V.tensor_tensor(m1[:], m1[:], m2[:], A.add)
                    comb = opool.tile([128, jchunk, 64], BF16, tag="comb")
                    V.tensor_tensor(comb[:], m1[:, :, 0, :], m1[:, :, 1, :], A.add)
                    V.tensor_tensor(comb[:], comb[:], m1[:, :, 2, :], A.add)

                    # transpose [pt, c] -> [c, pt]: J/2 transposes of [128,128]
                    ps = opsum.tile([128, (jchunk // 2) * 128], BF16, tag="psO")
                    combv = comb[:].rearrange("p j c -> p (j c)")
                    for t in range(jchunk // 2):
                        nc.tensor.transpose(
                            ps[:, t * 128 : (t + 1) * 128],
                            combv[:, t * 128 : (t + 1) * 128],
                            identb[:],
                        )
                    # psum [(j&1)*64+c, (j>>1)*128+pt] -> outsb [c, j*128+pt]
                    outsb = opool.tile([C, jchunk * 128], F32, tag="outsb")
                    psv = ps[:].rearrange("p (t q) -> p t q", q=128)
                    osv = outsb[:].rearrange("c (j q) -> c j q", q=128)
                    S.copy(osv[:, 0::2, :], psv[0:64, :, :])
                    S.copy(osv[:, 1::2, :], psv[64:128, :, :])
                    iy0 = (ci // TGRID_X) * TILE_IY
                    ix0 = (ci % TGRID_X) * TILE_IX
                    dsto = AP(
                        ob.tensor,
                        ob.offset + iy0 * W + ix0,
                        [[HW, C], [W, TILE_IY], [1, TILE_IX]],
                    )
                    nc.sync.dma_start(out=dsto, in_=outsb[:])
                yield "E"

            for _rep in range(reps):
                gens = [batch_phases(b) for b in range(n_batches)]
                if order == "interleave" and n_batches == 2:
                    g0, g1 = gens
                    next(g0)        # A0
                    next(g0)        # CD0
                    next(g1)        # A1
                    for _ in g0:    # E0
                        pass
                    for _ in g1:    # CD1, E1
                        pass
                else:
                    for g_ in gens:
                        for _ in g_:
                            pass

    nc.compile()
    return nc


_CONSTS = _host_consts()


def _batch_perm(flags):
    """Pair heavy with light batches: perm[2k], perm[2k+1] -> core k."""
    loads = flags.sum(axis=1)
    order = np.argsort(-loads, kind="stable")
    perm = []
    for k in range(NCORES):
        perm.append(int(order[k]))
        perm.append(int(order[2 * NCORES - 1 - k]))
    return perm


def _make_in_maps(X, theta, n_batches=BPC, perm=None):
    Xr = np.ascontiguousarray(X.reshape(B, C, HW), np.float32)
    th = np.ascontiguousarray(theta, np.float32)
    flags = _host_chunk_flags(th)  # [B, NCHUNK] int32
    cnts = np.where(flags > 0, CHUNK_PTS, 16).astype(np.int32)
    if perm is None:
        perm = list(range(B))
    in_maps = []
    for core in range(NCORES):
        bsel = perm[core * n_batches : (core + 1) * n_batches]
        th_rep = np.repeat(th[bsel][:, None, :], 128, axis=1)
        chf = np.repeat(flags[bsel][:, None, :], 128, axis=1)
        in_maps.append(
            {
                "X": np.ascontiguousarray(Xr[bsel]),
                "THETA": np.ascontiguousarray(th_rep, np.float32),
                "CHFLAG": np.ascontiguousarray(chf, np.int32),
                "CHCNT": np.ascontiguousarray(cnts[bsel][:, None, :], np.int32),
                **_CONSTS,
            }
        )
    return in_maps


_NC_CACHE = {}


def kernel(X, affine_transformation):
    from concourse.bass_utils import run_bass_kernel_spmd

    X = np.asarray(X, np.float32)
    theta = np.asarray(affine_transformation, np.float32)
    if "nc" not in _NC_CACHE:
        _NC_CACHE["nc"] = build_nc()
    nc = _NC_CACHE["nc"]
    flags = _host_chunk_flags(theta)
    perm = _batch_perm(flags)
    in_maps = _make_in_maps(X, theta, perm=perm)
    res = run_bass_kernel_spmd(nc, in_maps, list(range(NCORES)))
    outs = np.concatenate(
        [r["OUT"].reshape(BPC, C, H, W) for r in res.results], axis=0
    )
    full = np.empty_like(outs)
    full[perm] = outs
    return full


if __name__ == "__main__":
    mode = sys.argv[1] if len(sys.argv) > 1 else "sim"
    if mode == "build":
        nc = build_nc()
        print("build ok")
    elif mode == "sim":
        n_chunks = int(sys.argv[2]) if len(sys.argv) > 2 else 2
        import concourse.bass_interp as bass_interp

        _orig_copy = bass_interp.InstructionExecutor.visit_InstTensorCopy

        def _copy_rne(self, instruction, *, reg_snapshot=None):
            from concourse.bass_interp import Direction, InterpAPClass

            inp, outp = instruction.ins[0], instruction.outs[0]
            if isinstance(inp, InterpAPClass) and isinstance(outp, InterpAPClass):
                iv = self.view_ap(
                    inp, Direction.READ, instruction, reg_snapshot=reg_snapshot
                )
                ov = self.view_ap(
                    outp, Direction.WRITE, instruction, reg_snapshot=reg_snapshot
                )
                if np.issubdtype(iv.dtype, np.floating) and np.issubdtype(
                    ov.dtype, np.integer
                ):
                    ov[:] = np.round(iv.reshape(ov.shape))
                    return
            return _orig_copy(self, instruction, reg_snapshot=reg_snapshot)

        bass_interp.InstructionExecutor.visit_InstTensorCopy = _copy_rne

        rng = np.random.default_rng(0)
        Xt = rng.standard_normal((1, C, HW), dtype=np.float32)
        th = rng.standard_normal((1, 6), dtype=np.float32) * 0.7
        nc = build_nc(n_batches=1, n_chunks=n_chunks, debug=False, order="seq")
        th_rep = np.repeat(th[:, None, :], 128, axis=1)
        sim = bass_interp.CoreSim(nc)
        sim.tensor("X")[:] = Xt
        sim.tensor("THETA")[:] = np.ascontiguousarray(th_rep, np.float32)
        flags = _host_chunk_flags(th)
        cnts = np.where(flags > 0, CHUNK_PTS, 16).astype(np.int32)
        sim.tensor("CHFLAG")[:] = np.repeat(flags[:, None, :], 128, axis=1)
        sim.tensor("CHCNT")[:] = cnts[:, None, :]
        for k, v in _CONSTS.items():
            sim.tensor(k)[:] = v
        sim.simulate()
        got = np.array(sim.tensor("OUT"))

        def ref(Xf, thf):
            xl = np.linspace(-1, 1, W, dtype=np.float32)
            yl = np.linspace(-1, 1, H, dtype=np.float32)
            xc, yc = np.meshgrid(xl, yl, indexing="ij")
            grid = np.stack([xc.ravel(), yc.ravel(), np.ones(W * H, np.float32)], 0)
            thr = thf.reshape(-1, 2, 3)
            sampled = np.einsum("bij,jn->bin", thr, grid)
            x = (sampled[:, 0, :] + 1) * (W * 0.5)
            y = (sampled[:, 1, :] + 1) * (H * 0.5)
            x0 = np.clip(np.floor(x).astype(np.int64), 0, W - 1)
            x1 = np.clip(np.floor(x).astype(np.int64) + 1, 0, W - 1)
            y0 = np.clip(np.floor(y).astype(np.int64), 0, H - 1)
            y1 = np.clip(np.floor(y).astype(np.int64) + 1, 0, H - 1)
            flat = Xf.reshape(-1, C, H * W).transpose(0, 2, 1)
            bidx = np.arange(flat.shape[0])[:, None]
            pa = flat[bidx, y0 * W + x0]
            pb = flat[bidx, y1 * W + x0]
            pc = flat[bidx, y0 * W + x1]
            pd = flat[bidx, y1 * W + x1]
            x0f, x1f, y0f, y1f = (a.astype(np.float32) for a in (x0, x1, y0, y1))
            wa = ((x1f - x) * (y1f - y))[..., None]
            wb = ((x1f - x) * (y - y0f))[..., None]
            wc = ((x - x0f) * (y1f - y))[..., None]
            wd = ((x - x0f) * (y - y0f))[..., None]
            out = wa * pa + wb * pb + wc * pc + wd * pd
            return out.reshape(-1, W, H, C).transpose(0, 3, 2, 1)

        exp_full = ref(Xt, th).reshape(1, C, H, W)
        gotr = got[0].reshape(C, H, W)
        print("chunk flags:", flags[0][:n_chunks], "...")
        gs, es = [], []
        for ci in range(n_chunks):
            iy0 = (ci // TGRID_X) * TILE_IY
            ix0 = (ci % TGRID_X) * TILE_IX
            gs.append(gotr[:, iy0 : iy0 + TILE_IY, ix0 : ix0 + TILE_IX].ravel())
            es.append(
                exp_full[0][:, iy0 : iy0 + TILE_IY, ix0 : ix0 + TILE_IX].ravel()
            )
        got_s = np.concatenate(gs)
        exp_s = np.concatenate(es)
        err = np.abs(got_s - exp_s)
        denom = np.abs(exp_s).max() + 1e-8
        print("max abs err:", err.max(), " max |exp|:", np.abs(exp_s).max())
        print(
            "rel l2:",
            np.linalg.norm(got_s - exp_s) / (np.linalg.norm(exp_s) + 1e-8),
        )
        bad = np.argwhere(err > 3e-2 * denom)
        print("n bad:", len(bad), "of", got_s.size)
        if len(bad):
            print("first bad:", bad[:5])
